# revision 2
# baseline (speedup 1.0000x reference)
"""Trainium2 Bass kernel for the pairwise+triplewise cycle-consistency loss.

v2 strategy (8 cores, tensor-parallel over rows of each [N,N] block):
  - A-side logits matmuls in bf16 (full-rate, half the DMA of f32).
  - M-product (S12_hat @ S21_hat) in fp8e4 with DoubleRow perf mode
    (2x PE throughput); softmax outputs quantized to fp8 (rel err ~2e-4).
  - S21_hat AllGathers batched: ONE collective for the 3 pair terms and
    ONE for the 3 triple terms (bigger transfers hit the high-bw regime,
    and only 2 collectives serialize on the collective cores).
  - Grams (G_k = nf_k^T nf_k, for the triple terms) computed FULLY on
    every core from a token-major bf16 copy of nf_k (no AllReduce at all).
  - Pipeline: pair AT-sides -> kick AG_pairs -> grams -> tri AT-sides ->
    kick AG_tris -> per term (A-side -> M-phase) with the AGs in flight.
  - M-phase stats (rowmax/colmax/diag) batched over 4 column-tiles per
    DVE op; diag selected via the per-core one-hot `wsel` input.
"""
import sys
sys.path.insert(0, "/opt/trn_rl_repo")

import math
import numpy as np

import concourse.bass as bass
import concourse.mybir as mybir
import concourse.tile as tile
from concourse import bacc
from concourse.bass_utils import run_bass_kernel_spmd
from concourse.masks import make_identity

F32 = mybir.dt.float32
BF16 = mybir.dt.bfloat16
FP8 = mybir.dt.float8e4
AX = mybir.AxisListType
OP = mybir.AluOpType
ACT = mybir.ActivationFunctionType
DR = mybir.MatmulPerfMode.DoubleRow

NTOK = 4096          # rows per view
D = 1024             # feature dim
NC = 8               # cores
RPC = NTOK // NC     # rows per core (512)
P = 128
NRT = RPC // P       # rowtiles per core (4)
NS = 8               # 512-col strips of A
DKB = D // P         # d-blocks (8)
NKB = NTOK // P      # k-blocks of the M product (32)
NJG = 8              # jtile groups (4 jtiles each) in M product
SCALE = math.log(NTOK) / 0.1
MARGIN = 0.5

# term table: (is_tri, gram_idx, lhsA, rhsA, lhsAT, rhsAT); lhs names index
# xb_i/ut, rhs names index f_i. For tri terms lhs is G[gram_idx] @ x_i.
TERMS = [
    (False, None, 0, 1, 1, 0),   # S01
    (False, None, 0, 2, 2, 0),   # S02
    (False, None, 1, 2, 2, 1),   # S12
    (True, 2, 0, 1, 1, 0),       # S02 @ S21 = nf0 G2 nf1^T
    (True, 1, 0, 2, 2, 0),       # S01 @ S12 = nf0 G1 nf2^T
    (True, 0, 1, 2, 2, 1),       # S10 @ S02 = nf1 G0 nf2^T
]

OUT_W = RPC + 32 + NRT   # racc 512 | colmax32 32 | diag 4


def build_program():
    nc = bacc.Bacc("TRN2", target_bir_lowering=False, debug=False, num_devices=NC)

    xbs = [nc.dram_tensor(f"xb{i}", [D, RPC], BF16, kind="ExternalInput")
           for i in range(3)]
    fs = [nc.dram_tensor(f"f{i}", [D, NTOK], BF16, kind="ExternalInput")
          for i in range(3)]
    wfs = [nc.dram_tensor(f"wf{i}", [NTOK, D], BF16, kind="ExternalInput")
           for i in range(3)]
    wsel_in = nc.dram_tensor("wsel", [P, P], F32, kind="ExternalInput")
    out = nc.dram_tensor("out", [6, P, OUT_W], F32, kind="ExternalOutput")

    with tile.TileContext(nc) as tc:
        with (
            tc.tile_pool(name="cst", bufs=1) as cst,
            tc.tile_pool(name="lhs", bufs=2) as lhsp,
            tc.tile_pool(name="rhs", bufs=2) as rhsp,
            tc.tile_pool(name="aq", bufs=4) as aqp,
            tc.tile_pool(name="pt", bufs=2) as ptp,
            tc.tile_pool(name="qbf", bufs=2) as qbfp,
            tc.tile_pool(name="qc", bufs=1) as qcp,
            tc.tile_pool(name="qsb", bufs=3) as qsbp,
            tc.tile_pool(name="st", bufs=2) as stp,
            tc.tile_pool(name="msb", bufs=1) as msbp,
            tc.tile_pool(name="tmp", bufs=2) as tmpp,
            tc.tile_pool(name="sm", bufs=4) as smp,
            tc.tile_pool(name="psA", bufs=2, space="PSUM") as psA,
            tc.tile_pool(name="psT", bufs=2, space="PSUM") as psT,
            tc.tile_pool(name="psM", bufs=4, space="PSUM") as psM,
            tc.tile_pool(name="dram", bufs=1, space="DRAM") as dram,
            tc.tile_pool(name="dram2", bufs=2, space="DRAM") as dram2,
        ):
            # ---------------- constants ----------------
            identb = cst.tile([P, P], BF16)
            make_identity(nc, identb)
            wsel = cst.tile([P, P], F32)
            nc.sync.dma_start(wsel[:], wsel_in[:])
            nwsel = cst.tile([P, P], F32)
            nc.vector.tensor_scalar_mul(nwsel[:], wsel[:], -1.0)
            # view [p, jg, j2, b]
            nwselG = nwsel.rearrange("p (g a b) -> p g a b", a=NRT, b=NRT)
            # imask4[p, 128b+p] = 1 for b in 0..3 (diag candidate positions)
            imask4 = cst.tile([P, NRT, P], F32)
            identf = cst.tile([P, P], F32)
            make_identity(nc, identf)
            for b in range(NRT):
                nc.vector.tensor_copy(imask4[:, b, :], identf[:])
            # resident bf16 lhs blocks (d-major rows of nf^T for this core)
            xb_sb = []
            for i in range(3):
                t = cst.tile([P, DKB, RPC], BF16, name=f"xbs{i}")
                nc.sync.dma_start(t[:], xbs[i].rearrange("(o p) r -> p o r", p=P))
                xb_sb.append(t)

            # gram outputs in DRAM (bf16, local to each core)
            gdram = [dram.tile([D, D], BF16, tag=f"g{k}", name=f"g{k}")
                     for k in range(3)]
            # allgather buffers: batch 0 = pair terms, batch 1 = tri terms
            ag_in = [dram2.tile([3, RPC, NTOK], FP8, tag="agin", name=f"agin{b}")
                     for b in range(2)]
            ag_out = [dram2.tile([NC, 3, RPC, NTOK], FP8, tag="agout",
                                 addr_space="Shared", name=f"agout{b}")
                      for b in range(2)]

            # ---------------- helpers ----------------
            def compute_gram(k):
                """G_k = wf_k^T @ wf_k, full, on every core. bf16 -> gdram[k]."""
                for d1h in range(2):
                    for d2h in range(2):
                        gps = [psM.tile([P, 512], F32, tag="psM",
                                        name=f"gps{k}_{d1h}_{d2h}_{q}")
                               for q in range(4)]
                        for c in range(8):
                            wfc = rhsp.tile([P, 4, D], BF16, tag="rhs",
                                            name=f"wfc{k}_{d1h}_{d2h}_{c}")
                            nc.sync.dma_start(
                                wfc[:], wfs[k][c * 512:(c + 1) * 512, :]
                                .rearrange("(o p) d -> p o d", p=P))
                            for o in range(4):
                                for q in range(4):
                                    d1 = d1h * 4 + q
                                    nc.tensor.matmul(
                                        gps[q][:],
                                        wfc[:, o, d1 * P:(d1 + 1) * P],
                                        wfc[:, o, d2h * 512:(d2h + 1) * 512],
                                        start=(c == 0 and o == 0),
                                        stop=(c == 7 and o == 3))
                        for q in range(4):
                            d1 = d1h * 4 + q
                            gt = stp.tile([P, 512], BF16, tag="gt",
                                          name=f"gt{k}_{d1h}_{d2h}_{q}")
                            nc.scalar.copy(gt[:], gps[q][:])
                            nc.sync.dma_start(
                                gdram[k][d1 * P:(d1 + 1) * P,
                                         d2h * 512:(d2h + 1) * 512], gt[:])

            def compute_ut(gk, i, nm):
                """U^T[:, R_c] = G_k @ x_i  -> [128, DKB, RPC] bf16 tile."""
                ut = lhsp.tile([P, DKB, RPC], BF16, tag="lhs", name=f"ut_{nm}")
                for grp in range(2):
                    pss = [psM.tile([P, 512], F32, tag="psM",
                                    name=f"utps_{nm}_{grp}_{d4}")
                           for d4 in range(4)]
                    for half in range(2):
                        gh = rhsp.tile([P, 4, D], BF16, tag="rhs",
                                       name=f"gh_{nm}_{grp}_{half}")
                        nc.sync.dma_start(
                            gh[:], gdram[gk][half * 512:(half + 1) * 512, :]
                            .rearrange("(o p) d -> p o d", p=P))
                        for d4 in range(4):
                            dp = 4 * grp + d4
                            for db in range(4):
                                nc.tensor.matmul(
                                    pss[d4][:], gh[:, db, dp * P:(dp + 1) * P],
                                    xb_sb[i][:, 4 * half + db, :],
                                    start=(half == 0 and db == 0),
                                    stop=(half == 1 and db == 3))
                    for d4 in range(4):
                        nc.scalar.copy(ut[:, 4 * grp + d4, :], pss[d4][:])
                return ut

            def side_chunk(lhs_t, fj, nm):
                """A-side chunk [RPC, 4096] raw logits in 4 quarter tiles (f32)."""
                chunk = [aqp.tile([P, NTOK], F32, tag="aq", name=f"ch_{nm}_{rt}")
                         for rt in range(NRT)]
                for s in range(NS):
                    rsb = rhsp.tile([P, DKB, 512], BF16, tag="rhs",
                                    name=f"rs_{nm}_{s}")
                    nc.sync.dma_start(
                        rsb[:], fs[fj][:, s * 512:(s + 1) * 512]
                        .rearrange("(o p) n -> p o n", p=P))
                    for rt in range(NRT):
                        ps = psA.tile([P, 512], F32, tag="psA",
                                      name=f"aps_{nm}_{s}_{rt}")
                        for kb in range(DKB):
                            nc.tensor.matmul(
                                ps[:], lhs_t[:, kb, rt * P:(rt + 1) * P],
                                rsb[:, kb, :], start=(kb == 0),
                                stop=(kb == DKB - 1))
                        nc.scalar.copy(chunk[rt][:, s * 512:(s + 1) * 512], ps[:])
                return chunk

            def softmax_quarter(q, nm):
                """in-place exp(SCALE*(x - rowmax)); returns reciprocal row sum."""
                rm = smp.tile([P, 1], F32, tag="sm", name=f"rm_{nm}")
                nc.vector.reduce_max(rm[:], q[:], axis=AX.X)
                bias = smp.tile([P, 1], F32, tag="sm", name=f"bias_{nm}")
                nc.vector.tensor_scalar_mul(bias[:], rm[:], -SCALE)
                ssum = smp.tile([P, 1], F32, tag="sm", name=f"ss_{nm}")
                nc.scalar.activation(q[:], q[:], ACT.Exp, bias=bias[:], scale=SCALE,
                                     accum_out=ssum[:])
                rs = smp.tile([P, 1], F32, tag="sm", name=f"rs_{nm}")
                nc.vector.reciprocal(rs[:], ssum[:])
                return rs

            def at_phase(t, lhs_t, fj):
                """A^T side: softmax rows -> fp8 -> stage into ag_in batch slot."""
                b, s = (0, t) if t < 3 else (1, t - 3)
                chunk = side_chunk(lhs_t, fj, f"at{t}")
                for rt in range(NRT):
                    rq = softmax_quarter(chunk[rt], f"at{t}_{rt}")
                    qb = qbfp.tile([P, NTOK], FP8, tag="qbf", name=f"qb{t}_{rt}")
                    nc.scalar.activation(qb[:], chunk[rt][:], ACT.Copy, bias=0.0,
                                         scale=rq[:])
                    nc.sync.dma_start(ag_in[b][s, rt * P:(rt + 1) * P, :], qb[:])

            def kick_ag(b):
                nc.gpsimd.collective_compute(
                    "AllGather", OP.bypass, replica_groups=[list(range(NC))],
                    ins=[ag_in[b][:]], outs=[ag_out[b][:]])

            def a_phase(t, lhs_t, fj):
                """A side: softmax, normalize bf16, transpose into fp8 PT tile."""
                pt = ptp.tile([P, NKB, RPC], FP8, tag="pt", name=f"pt{t}")
                chunk = side_chunk(lhs_t, fj, f"a{t}")
                for rt in range(NRT):
                    rp = softmax_quarter(chunk[rt], f"a{t}_{rt}")
                    qc = qcp.tile([P, NTOK], BF16, tag="qc", name=f"qc{t}_{rt}")
                    nc.scalar.activation(qc[:], chunk[rt][:], ACT.Copy,
                                         bias=0.0, scale=rp[:])
                    for j in range(NKB):
                        tp = psT.tile([P, P], BF16, tag="psT",
                                      name=f"tp{t}_{rt}_{j}")
                        nc.tensor.transpose(tp[:], qc[:, j * P:(j + 1) * P],
                                            identb[:])
                        nc.vector.tensor_copy(pt[:, j, rt * P:(rt + 1) * P], tp[:])
                return pt

            def m_phase(t, pt):
                """M^T tiles = (S12_hat @ S21_hat)^T[jtile, R_c]; stats to out[t].

                fp8 DoubleRow matmuls, stats batched over the 4 jtiles of each
                jg group. Diag candidates live at (p, j2, b, q=p); the one-hot
                wsel input (j == 4c+b) selects the true diagonal per core.
                """
                b, s = (0, t) if t < 3 else (1, t - 3)
                racc = stp.tile([P, RPC], F32, tag="racc", name=f"racc{t}")
                nc.vector.memset(racc[:], 0.0)
                diagacc = smp.tile([P, NRT], F32, tag="dac", name=f"dac{t}")
                nc.vector.memset(diagacc[:], 0.0)
                cm32 = stp.tile([P, 32], F32, tag="cm32", name=f"cm32{t}")
                for jg in range(NJG):
                    pss = [psM.tile([P, 512], F32, tag="psM",
                                    name=f"mps{t}_{jg}_{j2}")
                           for j2 in range(4)]
                    for kb2 in range(NKB // 2):
                        c, r0 = kb2 // 2, (kb2 % 2) * 256
                        qsb = qsbp.tile([P, 2, 512], FP8, tag="qsb",
                                        name=f"qs{t}_{jg}_{kb2}")
                        nc.sync.dma_start(
                            qsb[:], ag_out[b][c, s, r0:r0 + 256,
                                              jg * 512:(jg + 1) * 512]
                            .rearrange("(o p) n -> p o n", p=P))
                        for j2 in range(4):
                            nc.tensor.matmul(
                                pss[j2][:], qsb[:, 0:2, j2 * P:(j2 + 1) * P],
                                pt[:, 2 * kb2:2 * kb2 + 2, :],
                                start=(kb2 == 0), stop=(kb2 == NKB // 2 - 1),
                                perf_mode=DR)
                    msb4 = msbp.tile([P, 4, 512], F32, tag="msb",
                                     name=f"msb{t}_{jg}")
                    for j2 in range(4):
                        nc.scalar.copy(msb4[:, j2, :], pss[j2][:])
                    m4 = msb4.rearrange("p a (b q) -> p a b q", q=P)
                    tmp = tmpp.tile([P, 4, NRT, P], F32, tag="tmp",
                                    name=f"t4_{t}_{jg}")
                    im4b = imask4[:, None, :, :].to_broadcast((P, 4, NRT, P))
                    nc.vector.tensor_tensor(tmp[:], m4[:], im4b, op=OP.mult)
                    dv16 = smp.tile([P, 4, NRT], F32, tag="sm16",
                                    name=f"dv_{t}_{jg}")
                    nc.vector.reduce_sum(dv16[:], tmp[:], axis=AX.X)
                    dv16w = smp.tile([P, 4, NRT], F32, tag="sm16",
                                     name=f"dvw_{t}_{jg}")
                    nc.vector.tensor_tensor(dv16w[:], dv16[:], nwselG[:, jg],
                                            op=OP.mult)
                    for j2 in range(4):
                        nc.vector.tensor_add(diagacc[:], diagacc[:],
                                             dv16w[:, j2, :])
                    sc = tmpp.tile([P, 4, NRT, P], F32, tag="tmp",
                                   name=f"sc_{t}_{jg}")
                    nc.vector.tensor_tensor(
                        sc[:], im4b,
                        dv16w[:, :, :, None].to_broadcast((P, 4, NRT, P)),
                        op=OP.mult)
                    nc.vector.tensor_add(m4[:], m4[:], sc[:])
                    nc.vector.reduce_max(cm32[:, jg * 4:(jg + 1) * 4], msb4[:],
                                         axis=AX.X)
                    for j2 in range(4):
                        nc.vector.tensor_tensor(racc[:], racc[:], msb4[:, j2, :],
                                                op=OP.max)
                diag = smp.tile([P, NRT], F32, tag="dac", name=f"diag{t}")
                nc.vector.tensor_scalar_mul(diag[:], diagacc[:], -1.0)
                nc.sync.dma_start(out[t, :, 0:RPC], racc[:])
                nc.sync.dma_start(out[t, :, RPC:RPC + 32], cm32[:])
                nc.sync.dma_start(out[t, :, RPC + 32:OUT_W], diag[:])

            # ---------------- main pipeline ----------------
            # 1) pair AT sides, then kick AG for the pair batch
            for t in range(3):
                _, _, _, _, lat, rat = TERMS[t]
                at_phase(t, xb_sb[lat], rat)
            kick_ag(0)
            # 2) grams (no collectives), then tri AT sides, kick tri AG
            for k in (2, 1, 0):
                compute_gram(k)
            for t in range(3, 6):
                _, gk, _, _, lat, rat = TERMS[t]
                ut = compute_ut(gk, lat, f"at{t}")
                at_phase(t, ut, rat)
            kick_ag(1)
            # 3) software-pipelined: A(t+1) issued before M(t) so the PE
            # always has AG-independent work while a gather is in flight.
            prev = None
            for t, (is_tri, gk, la, ra, _, _) in enumerate(TERMS):
                if is_tri:
                    lhs_a = compute_ut(gk, la, f"a{t}")
                else:
                    lhs_a = xb_sb[la]
                pt = a_phase(t, lhs_a, ra)
                if prev is not None:
                    m_phase(*prev)
                prev = (t, pt)
            m_phase(*prev)

    nc.finalize()
    return nc


_PROGRAM = None


def _get_program():
    global _PROGRAM
    if _PROGRAM is None:
        _PROGRAM = build_program()
    return _PROGRAM


def _normalize(x):
    n = np.linalg.norm(x.astype(np.float32), axis=-1, keepdims=True)
    return (x / np.maximum(n, 1e-12)).astype(np.float32)


def _build_in_maps(inputs):
    import ml_dtypes
    BF = ml_dtypes.bfloat16
    nf = [_normalize(np.asarray(inputs[k], np.float32))
          for k in ("feat0", "feat1", "feat2")]
    nfT_b = [np.ascontiguousarray(x.T).astype(BF) for x in nf]
    nf_b = [np.ascontiguousarray(x.astype(BF)) for x in nf]

    in_maps = []
    for c in range(NC):
        rows = slice(c * RPC, (c + 1) * RPC)
        m = {}
        for i in range(3):
            m[f"xb{i}"] = np.ascontiguousarray(nfT_b[i][:, rows])
            m[f"f{i}"] = nfT_b[i]
            m[f"wf{i}"] = nf_b[i]
        wsel = np.zeros((P, P), np.float32)
        for b in range(NRT):
            j = 4 * c + b
            wsel[:, 4 * j + b] = 1.0     # [p, jg, j2, b] one-hot layout
        m["wsel"] = wsel
        in_maps.append(m)
    return in_maps


def _reduce(results):
    """results: list (per core) of {'out': [6, 128, OUT_W]} -> scalar loss."""
    L = np.zeros(6, np.float64)
    for t in range(6):
        rowpart = 0.0
        colmax = np.full(NTOK, -np.inf)
        diag_g = np.zeros(NTOK)
        for c in range(NC):
            o = results[c]["out"][t].astype(np.float64)
            racc = o[:, 0:RPC]
            cm32 = o[:, RPC:RPC + 32]
            dacc = o[:, RPC + 32:OUT_W]
            rowmax_local = racc.max(axis=0)                   # [512]
            diag_local = dacc.T.reshape(RPC)                  # [512]
            rowpart += np.maximum(rowmax_local + MARGIN - diag_local, 0.0).sum()
            colmax = np.maximum(colmax, cm32.T.reshape(NTOK))
            diag_g[c * RPC:(c + 1) * RPC] = diag_local
        colpart = np.maximum(colmax + MARGIN - diag_g, 0.0).sum()
        L[t] = (rowpart + colpart) / (2.0 * NTOK)
    loss = (L[0] + L[1] + L[2]) / 3.0 + (L[3] + L[4] + L[5]) / 3.0
    return np.float32(loss)


def kernel(feat0, feat1, feat2):
    in_maps = _build_in_maps({"feat0": feat0, "feat1": feat1, "feat2": feat2})
    nc = _get_program()
    res = run_bass_kernel_spmd(nc, in_maps, core_ids=list(range(NC)))
    return _reduce(res.results)


if __name__ == "__main__":
    rng = np.random.default_rng(0)
    f0 = rng.standard_normal((NTOK, D), dtype=np.float32)
    f1 = rng.standard_normal((NTOK, D), dtype=np.float32)
    f2 = rng.standard_normal((NTOK, D), dtype=np.float32)
    print("loss:", kernel(f0, f1, f2))
